# revision 56
# baseline (speedup 1.0000x reference)
"""C2Q (BiDAF-style) attention kernel for 8 TRN2 NeuronCores (v5, HW-tuned).

Pure data parallel: 64 batches sharded 8-per-core. Per batch b (reference):
    S = c @ c_w + (q @ q_w)^T + (c * cq_w) @ q^T + bias      (1024, 128)
    S1 = masked_softmax(S, q_mask, axis=j)
    S2 = masked_softmax(S1, c_mask, axis=i)
    A = S1 @ q ; Bm = S1 @ (S2^T @ c)
    out = [c | A | c*A | c*Bm]                                (1024, 512)

Key algebra: softmax over j is invariant to per-i constants, so c @ c_w
and the bias cancel in S1. Only R[j] = q @ q_w + log-mask(q_mask)
survives as a per-partition bias in the transposed domain.

v2/v3 structure (aimed at the DMA, HWDGE-per-DMA and ACT/DVE rooflines):
  * Device writes ONLY [A | c*A | c*Bm] in fp16 (6MB/core instead of
    16MB f32); the host pastes the exact f32 `c` block and upcasts.
  * c_mask folded multiplicatively: rcprowm = rcprow * m zeroes masked
    rows of S1 BEFORE the second exp, making them exp(0)=1; an exact
    rank-1 correction (ones ⊗ K, K = host-computed masked-row sums of
    [c|1]) is subtracted inside the Traw PSUM accumulation. This keeps
    ONE unmasked cN slab for both the Traw matmul and the elementwise
    c*A / c*Bm products, and drops the per-chunk bias from the G exp.
  * rowsum via 8 one-column PE matmuls (E0T_k^T @ ones).
  * ONE input DMA per batch ([qmod|ones|q|Ts|cT|cN] slab) and ONE
    output DMA per batch (rearranged AP) — HWDGE costs ~625ns per DMA
    instruction, so 19 DMAs/core instead of 91.
  * Stage-pipelined emission: block b emits stA(b+1) | abmm(b-1) woven
    with stB(b+1) | recip_g(b+1) | stC(b) | load(b+2), so the in-order
    PE queue fills ab-rotation waits with next-batch transposes/minis.
  * Output staging (Pool cannot touch PSUM on real HW): per chunk ONE
    psum evacuation with the rcprow scale (DVE, 2 chunks on ACT) writes
    [Bs | As] where As doubles as the A output block; then SBUF-only
    c*A on Pool and c*Bm on DVE (all-f16 4x mode). The out-DMA's 3D AP
    skips the Bs scratch column.
  * Timing loop uses For_i(staggered_reset=True): the inter-iteration
    semaphore reset is staggered instead of a full all-engine barrier,
    letting consecutive iterations overlap (~3.6us/iter on HW).

Device per batch:
    S^T[j,i] = qmodT.T @ cT    (f16, 2 matmuls of N=512)
    E0T      = exp(S^T + R[j])               # ACT bias; bf16 [j, 1024]
    ep_k     = transpose(E0T chunk)          # PE; bf16 psum [i, j]
    rowsum_k = E0T_k^T @ ones                # PE minis -> psum f32
    rcprow = 1/rowsum ; rcprowm = rcprow * cmask
    G_k = exp(ep_k * rcprowm_k)              # ACT scale AP; f16 SBUF
    Traw = -ones^T@K + sum_k G_k^T @ [c_k|1] # psum f32 accum [j, 129]
    Ts = Traw[:,0:128] * (1/Traw[:,128])     # -> f16, into slab
    ab_k = E0T_k^T @ [q | Ts]                # psum f32 [i, 256]
    st = [ab_A*r | c*ab_A*r | c*ab_B*r]      # f16, one DMA per batch
No max-subtraction needed: |S+R| <= ~30 so exp stays in range.
"""

import os
import numpy as np
import ml_dtypes

import concourse.bass as bass
import concourse.tile as tile
from concourse import bacc, mybir
from concourse.bass_utils import run_bass_kernel_spmd

F32 = mybir.dt.float32
F16 = mybir.dt.float16
BF16 = mybir.dt.bfloat16
AF = mybir.ActivationFunctionType
ALU = mybir.AluOpType

N_CORES = 8
B, CL, QL, D = 64, 1024, 128, 128
BPC = B // N_CORES          # batches per core
NK = CL // 128              # 128-row chunks per batch
MASK_NEG = -50.0            # exp(-50+eps) vanishes in f32 sums; in ACT range

# input slab column layout (f16): [qmod | ones | cT | Ts | q | pad | cN].
# One leading DMA covers everything stA needs ([qmod|ones|cT]); Ts is
# device-written scratch (never transferred); [Ts|q] stay adjacent for the
# ab matmul's 256-col moving operand. cN chunks padded to 130 cols so the
# per-chunk c base stays 4B-aligned (keeps DVE/Pool tensor ops in 2x mode
# on HW, whose auto-detect requires packed+aligned reads).
QMOD0, ONES0, CT0, TS0, QROW0, CN0 = 0, 128, 129, 1153, 1281, 1410
CNS = 130                   # padded chunk stride: [c_k | 1 | pad]
SLAB = CN0 + NK * CNS       # 2450

# staging engine assignment knobs (tuned on HW); env-overridable for A/B
def _envset(name, default):
    v = os.environ.get(name)
    return frozenset(int(x) for x in v.split(",") if x != "") if v is not None else default


def _envint(name, default):
    v = os.environ.get(name)
    return int(v) if v is not None else default


EVAC_ACT = _envset("K_EVAC_ACT", frozenset({6, 7}))  # chunks whose evac runs on ACT
CBM_POOL = _envset("K_CBM_POOL", frozenset({0, 1}))  # chunks whose c*Bm runs on Pool
CA_ON_POOL = bool(_envint("K_CA_POOL", 1))         # c*A on Pool (else DVE)
ALT_FROM = _envint("K_ALT_FROM", BPC - 2)          # batches >= this alternate evacs
INTERLEAVE_STC = bool(_envint("K_IL_STC", 0))      # Traw matmuls inside the k-loop
SPLIT_IN_DMA = bool(_envint("K_SPLIT_IN", 1))      # 2 fat input DMAs (else 1)
OUT_PIECES = _envint("K_OUT_PIECES", 2)            # out-DMA pieces (last batch 2x)
FATOUT = bool(_envint("K_FATOUT", 0))              # contiguous [A|cA|cBm] out layout
                                                   # via 2-run evac AP (fat 6KB
                                                   # out-DMA descriptors)
SKIP_IN_DMA = False            # timing probe: skip slab loads
SKIP_OUT_DMA = False           # timing probe: skip out-DMA
NBATCH = BPC                   # timing probe: process only first N batches
STAGGER = True                 # For_i staggered reset (overlap loop iterations)
REPEAT = 1                     # sim probe: straight-line repeats of run_all

LAST_RESULTS = None         # set by kernel() for test.py profiling
M_VEC = [NK] * BPC          # per batch-slot: # of i-chunks with unmasked rows
PERM = None                 # (B, CL) row permutation used by _prep


def _build_graph(loop_n=0):
    """loop_n=0: straight-line graph (production). loop_n=N>0: wrap the whole
    computation in a hardware For_i loop repeating it N times (timing only)."""
    nc = bacc.Bacc()
    IN_LOOP = bool(loop_n)

    in_ext = nc.declare_dram_parameter("inp", [BPC, 128, SLAB], F16, isOutput=False)
    cmR_ext = nc.declare_dram_parameter("cmR", [128, BPC * NK + BPC], F32, isOutput=False)
    nko_ext = nc.declare_dram_parameter("nko", [1, BPC * 129 + 128], F16, isOutput=False)
    id_ext = nc.declare_dram_parameter("ident", [128, 128], BF16, isOutput=False)
    if FATOUT:
        # per-partition-contiguous layout [p, (k, 384)]; host unscrambles
        # (k,p)->row order inside its existing inverse-permutation gather
        out_ext = nc.declare_dram_parameter("out", [BPC, 128, NK * 3 * D], F16, isOutput=True)
    else:
        out_ext = nc.declare_dram_parameter("out", [BPC, CL, 3 * D], F16, isOutput=True)
    HEAD = CN0                  # head tile: [qmod | ones | Ts | q | cT]

    with tile.TileContext(nc) as tc:
        with (
            tc.tile_pool(name="const", bufs=1) as const,
            tc.tile_pool(name="inp", bufs=8) as inp,
            tc.tile_pool(name="e0tp", bufs=5) as e0tp,
            tc.tile_pool(name="gp", bufs=4) as gp,
            tc.tile_pool(name="small", bufs=6) as smallp,
            tc.tile_pool(name="stg", bufs=4) as stg,
            tc.tile_pool(name="stp", bufs=1, space=bass.MemorySpace.PSUM) as stp,
            tc.tile_pool(name="epp", bufs=2, space=bass.MemorySpace.PSUM) as epp,
            tc.tile_pool(name="rsp", bufs=1, space=bass.MemorySpace.PSUM) as rsp,
            tc.tile_pool(name="trawp", bufs=1, space=bass.MemorySpace.PSUM) as trawp,
            tc.tile_pool(name="abp", bufs=2, space=bass.MemorySpace.PSUM) as abp,
        ):
            ident = const.tile([128, 128], BF16, tag="ident")
            cmR = const.tile([128, BPC * NK + BPC], F32, tag="cmR")
            nko = const.tile([1, BPC * 129 + 128], F16, tag="nko")
            warm = const.tile([128, 1], F32, tag="warm")

            def act_warmup():
                # near-dep-free Exp so the ACT table set loads at t=0 instead
                # of stalling the first E0T behind a 1.3us LoadActFuncSet.
                nc.vector.memset(warm[:], 0.0)
                nc.scalar.activation(warm[:], warm[:], AF.Exp)

            def load_consts():
                # emitted after load(0) so batch 0's slab wins the first
                # HWDGE slots; cmR (E0T bias) before ident (transposes).
                nc.sync.dma_start(cmR[:], cmR_ext[:])
                nc.sync.dma_start(ident[:], id_ext[:])
                nc.sync.dma_start(nko[:], nko_ext[:])

            IN = {}
            CN = {}
            E0T = {}
            EP = {}
            RS = {}
            G = {}
            RCP = {}
            ST = {}
            AB2 = {}

            def load(b):
                # ONE slab DMA per batch: the dram block is contiguous, so
                # the DGE emits 128 fat descriptors (~4.9KB each). Splitting
                # this into skinnier pieces measured WORSE on HW (descriptor
                # overhead), even though it simulates faster.
                t = inp.tile([128, SLAB], F16, tag="in")
                if not SKIP_IN_DMA:
                    if SPLIT_IN_DMA:
                        nc.sync.dma_start(t[:, 0:TS0], in_ext[b][:, 0:TS0])
                        nc.sync.dma_start(t[:, QROW0:SLAB], in_ext[b][:, QROW0:SLAB])
                    else:
                        nc.sync.dma_start(t[:], in_ext[b][:])
                IN[b] = t

            def stA(b):
                """S^T + E0T exp."""
                t = IN[b]
                e0t = e0tp.tile([128, CL], BF16, tag="e0t")
                sp = stp.tile([128, CL], F32, tag="sp")
                for h in range(2):
                    nc.tensor.matmul(
                        sp[:, h * 512:(h + 1) * 512], t[:, QMOD0:QMOD0 + 128],
                        t[:, CT0 + h * 512:CT0 + (h + 1) * 512],
                    )
                nc.scalar.activation(
                    e0t[:], sp[:], AF.Exp,
                    bias=cmR[:, BPC * NK + b:BPC * NK + b + 1],
                )
                E0T[b] = e0t

            def transp(b, k):
                if k == 0:
                    EP[b] = epp.tile([128, CL], BF16, tag="ep", name="ep")
                if k >= M_VEC[b]:
                    return
                nc.tensor.transpose(
                    EP[b][:, k * 128:(k + 1) * 128],
                    E0T[b][:, k * 128:(k + 1) * 128], ident[:],
                )

            def mini(b, k):
                if k == 0:
                    RS[b] = rsp.tile([128, NK], F32, tag="rs", name="rs")
                nc.tensor.matmul(
                    RS[b][:, k:k + 1],
                    E0T[b][:, k * 128:(k + 1) * 128], IN[b][:, ONES0:ONES0 + 1],
                )

            def recip_g(b):
                """rcprow(+mask) and the M_b G exps (masked chunks dropped)."""
                rcprow = smallp.tile([128, NK], F32, tag="rcprow")
                rcprowm = smallp.tile([128, NK], F32, tag="rcprowm")
                nc.vector.reciprocal_approx_fast(rcprow[:], RS[b][:])
                nc.vector.tensor_mul(
                    rcprowm[:], rcprow[:], cmR[:, b * NK:(b + 1) * NK]
                )
                g = gp.tile([128, CL], F16, tag="g")
                for k in range(M_VEC[b]):
                    nc.scalar.activation(
                        g[:, k * 128:(k + 1) * 128],
                        EP[b][:, k * 128:(k + 1) * 128], AF.Exp,
                        scale=rcprowm[:, k:k + 1],
                    )
                G[b], RCP[b] = g, rcprow

            TRAW = {}

            def stC_init(b):
                """Rank-1 mask-fix matmul opens the Traw accumulation."""
                TRAW[b] = trawp.tile([128, 129], F32, tag="traw", name="traw")[:]
                nc.tensor.matmul(
                    TRAW[b], nko[0:1, BPC * 129:BPC * 129 + 128],
                    nko[0:1, b * 129:(b + 1) * 129],
                    start=True, stop=False,
                )

            def stC_k(b, k):
                if k >= M_VEC[b]:
                    return
                nc.tensor.matmul(
                    TRAW[b], G[b][:, k * 128:(k + 1) * 128],
                    IN[b][:, CN0 + k * CNS:CN0 + k * CNS + 129],
                    start=False, stop=(k == M_VEC[b] - 1),
                )

            def stC_fin(b):
                traw = TRAW[b]
                rcp2 = smallp.tile([128, 1], F32, tag="rcp2")
                nc.vector.reciprocal_approx_fast(rcp2[:], traw[:, 128:129])
                nc.vector.tensor_scalar_mul(
                    IN[b][:, TS0:TS0 + 128], traw[:, 0:128], rcp2[:]
                )

            def stC(b):
                """Traw accumulation (rank-1 mask fix first), then Ts."""
                stC_init(b)
                for k in range(NK):
                    stC_k(b, k)
                stC_fin(b)

            def abmm(b, k):
                """One AB matmul + staging; batched out-DMA on the last.

                ab = [Braw | Araw] (Ts precedes q in the slab). Staging per
                chunk: ONE psum evacuation with the rcprow scale (DVE/ACT;
                Pool cannot touch PSUM) -> st [Bs | As]; As doubles as the
                output A block. Then SBUF-only products c*A (Pool) and
                c*Bm (DVE 4x). Out-DMA skips the Bs scratch column.
                chunk st layout: [Bs | A | c*A | c*Bm] (512 cols)."""
                t, e0t, rcprow = IN[b], E0T[b], RCP[b]
                if k == 0:
                    ST[b] = stg.tile([128, NK * 512], F16, tag="st", name="st")
                st = ST[b]
                ab = abp.tile([128, 2 * QL], F32, tag="ab", name="ab")[:]
                nc.tensor.matmul(
                    ab, e0t[:, k * 128:(k + 1) * 128],
                    t[:, TS0:TS0 + 256],
                )
                cchunk = t[:, CN0 + k * CNS:CN0 + k * CNS + 128]
                evac_act = k in EVAC_ACT or (b >= ALT_FROM and k % 2 == 1)
                if FATOUT:
                    # chunk layout [A|cA|cBm] at k*384 (contiguous for a fat
                    # out-DMA: 128 x 6KB descriptors); Bs scratch banked at
                    # 3072+k*128. The evac dst is a handcrafted 2-run AP:
                    # src ab=[B|A] walks the Bs slot, then jumps (negative
                    # stride) back to the A slot.
                    a0, b0 = k * 384, NK * 384 + k * 128
                    dst = st[:, 2944:2944 + 256].rearrange(
                        "p (a b) -> p a b", b=128
                    ).copy()
                    dst.offset = dst.offset + (b0 - 2944)
                    dap = dst.ap
                    dap[1] = (a0 - b0, 2)
                    dst.ap = dap
                    sB, sA = b0, a0
                    sCA, sCB = a0 + 128, a0 + 256
                    esrc = ab.rearrange("p (a b) -> p a b", b=128)
                else:
                    s0 = k * 512
                    dst = st[:, s0:s0 + 256]
                    sB, sA = s0, s0 + 128
                    sCA, sCB = s0 + 256, s0 + 384
                    esrc = ab
                # evac psum with scale: st[Bs|As] = ab * rcprow_k
                if not evac_act:
                    nc.vector.tensor_scalar_mul(dst, esrc, rcprow[:, k:k + 1])
                else:
                    nc.scalar.activation(
                        dst, esrc, AF.Copy, scale=rcprow[:, k:k + 1],
                    )
                last = b == BPC - 1
                # c*A = As * c  (SBUF-only f16; Pool cannot touch PSUM but can
                # this). Last batch: alternate onto DVE — Pool otherwise
                # serializes the drain while DVE idles.
                ca_pool = CA_ON_POOL and not (last and k % 2 == 1)
                ca_eng = nc.gpsimd if ca_pool else nc.vector
                ca_eng.tensor_mul(
                    st[:, sCA:sCA + 128], st[:, sA:sA + 128], cchunk
                )
                # c*Bm = Bs * c  (SBUF-only f16; split Pool/DVE for balance)
                cbm_eng = nc.gpsimd if (k in CBM_POOL and not last) else nc.vector
                cbm_eng.tensor_mul(
                    st[:, sCB:sCB + 128], st[:, sB:sB + 128], cchunk
                )
                # out-DMA in pieces (2x finer for the last batch) so the
                # drain transfer starts as early as possible
                pieces = min(NK, OUT_PIECES * 2) if last else OUT_PIECES
                per = NK // pieces
                if (k + 1) % per == 0 and not SKIP_OUT_DMA:
                    if FATOUT:
                        c0, c1 = (k + 1 - per) * 384, (k + 1) * 384
                        nc.sync.dma_start(out_ext[b][:, c0:c1], st[:, c0:c1])
                    else:
                        r0, r1 = (k + 1 - per) * 128, (k + 1) * 128
                        nc.sync.dma_start(
                            out_ext[b][r0:r1].rearrange("(k p) d -> p k d", p=128),
                            st[:, (k + 1 - per) * 512:(k + 1) * 512]
                            .rearrange("p (k d) -> p k d", d=512)[:, :, 128:512],
                        )

            def run_all():
                # pipeline: block b emits stA(b+1) | abmm(b-1)⊗stB(b+1) |
                # recip_g(b+1) | stC(b) | load(b+2). In-order PE queue fills
                # ab-rotation and exp(b+1) waits with interleaved work.
                NB = NBATCH
                if not IN_LOOP:
                    act_warmup()
                load(0)
                if not IN_LOOP:
                    load_consts()
                if NB > 1:
                    load(1)
                if NB > 2:
                    load(2)
                stA(0)
                for k in range(NK):
                    transp(0, k)
                    mini(0, k)
                recip_g(0)
                for b in range(NB):
                    interleave = INTERLEAVE_STC or b + 1 >= NB
                    if b + 3 < NB:
                        load(b + 3)
                    if b + 1 < NB:
                        stA(b + 1)
                    for k in range(NK):
                        # Traw(b) interleaved into the k-loop: its PE matmuls
                        # ride along, pulling Ts(b) ~1us earlier so the next
                        # body's ab-chain (and its evacs) never starve.
                        if b >= 1:
                            abmm(b - 1, k)
                        if interleave:
                            if k == 0:
                                stC_init(b)
                            stC_k(b, k)
                        if b + 1 < NB:
                            transp(b + 1, k)
                            mini(b + 1, k)
                    if b + 1 < NB:
                        recip_g(b + 1)
                    if interleave:
                        stC_fin(b)
                    else:
                        stC(b)
                for k in range(NK):
                    abmm(NB - 1, k)

            if loop_n:
                # consts + ACT table warmup hoisted out of the timing loop:
                # they are iteration-invariant (3 DMAs + an exp per pass).
                act_warmup()
                load_consts()
                with tc.For_i(0, loop_n, 1, staggered_reset=STAGGER):
                    run_all()
            else:
                for _ in range(REPEAT):
                    run_all()
    return nc


def _prep(c, q, c_mask, q_mask, c_weight, q_weight, cq_weight, bias):
    global M_VEC, PERM
    c = np.ascontiguousarray(np.asarray(c, dtype=np.float32))
    q = np.ascontiguousarray(np.asarray(q, dtype=np.float32))
    c_mask = np.asarray(c_mask)
    q_mask = np.asarray(q_mask)
    q_weight = np.asarray(q_weight, dtype=np.float32)
    cq_weight = np.asarray(cq_weight, dtype=np.float32)
    f16 = np.float16

    # host-side prep (tiny). NOTE: c@c_weight and bias cancel in softmax_j.
    s1 = (q.reshape(-1, D) @ q_weight).reshape(B, QL)          # (B, 128)
    R = s1 + np.where(q_mask > 0, 0.0, MASK_NEG).astype(np.float32)
    cm0 = (c_mask > 0)                                         # (B, 1024)

    # Permute rows per batch: unmasked first. The second softmax only
    # involves unmasked rows, so the G/Traw chunk loops shrink to
    # M = ceil(max_unmasked/128) chunks (max across the 8 cores sharing
    # the graph); fully-masked chunks drop out exactly (their would-be
    # G=1 contribution and its rank-1 correction cancel). Exact for ANY
    # input since the graph is built after seeing the masks.
    PERM = np.argsort(~cm0, axis=1, kind="stable")             # (B, CL)
    bidx = np.arange(B)[:, None]
    c = np.ascontiguousarray(c[bidx, PERM])
    cm = cm0[bidx, PERM].astype(np.float32)
    n_unm = cm0.sum(axis=1).astype(np.int64)                   # (B,)
    per_slot = n_unm.reshape(N_CORES, BPC).max(axis=0)         # (BPC,)
    M_VEC = [max(1, min(NK, -(-int(n) // 128))) for n in per_slot]

    cT = c.transpose(0, 2, 1).astype(f16)                      # (B, 128, 1024)
    qmodT = (q * cq_weight.reshape(1, 1, D)).transpose(0, 2, 1).astype(f16)
    # cN: natural chunks [c_k | 1 | pad] -> (B, 128, NK*CNS)
    cNc = c.reshape(B, NK, 128, D).transpose(0, 2, 1, 3)       # (B, p, k, d)
    cN = np.concatenate(
        [cNc, np.ones((B, 128, NK, 1), np.float32),
         np.zeros((B, 128, NK, CNS - 129), np.float32)], axis=3
    ).astype(f16).reshape(B, 128, NK * CNS)
    slab = np.concatenate(
        [qmodT, np.ones((B, 128, 1), f16), cT, np.zeros((B, 128, 128), f16),
         q.astype(f16), np.zeros((B, 128, CN0 - QROW0 - 128), f16), cN],
        axis=2,
    )                                                          # (B, 128, SLAB)
    # rank-1 mask correction: K = [sum c | count] over the masked rows that
    # remain inside the first M chunks (rows beyond M*128 never enter Traw)
    w = (1.0 - cm)                                             # masked rows
    for b in range(B):
        mb = M_VEC[b % BPC]
        w[b, mb * 128:] = 0.0
    Kc = np.einsum('bi,bid->bd', w, c)                         # (B, 128)
    Kn = w.sum(axis=1)                                         # (B,)
    negK = -np.concatenate([Kc, Kn[:, None]], axis=1)          # (B, 129)

    in_maps = []
    for core in range(N_CORES):
        sl = slice(core * BPC, (core + 1) * BPC)
        cmN = cm[sl].reshape(BPC, NK, 128).transpose(2, 0, 1).reshape(128, BPC * NK)
        cmR = np.ascontiguousarray(
            np.concatenate([cmN, R[sl].T], axis=1)             # (128, 64+8)
        )
        nko = np.concatenate(
            [negK[sl].reshape(1, BPC * 129), np.ones((1, 128), np.float32)],
            axis=1,
        ).astype(f16)
        in_maps.append({
            "inp": np.ascontiguousarray(slab[sl]),
            "cmR": cmR,
            "nko": np.ascontiguousarray(nko),
            "ident": np.eye(128, dtype=ml_dtypes.bfloat16),
        })
    return in_maps


def make_in_maps():
    """For the local test/compare harness only (imports reference)."""
    import reference
    inputs = {k: np.asarray(v) for k, v in reference.setup_inputs().items()}
    return _prep(**inputs)


def _assemble(c, dev_out):
    """dev_out f16, rows in PERM order -> full (B, CL, 512) f32 with rows
    unpermuted and the exact c block pasted. FATOUT: dev_out is
    (B, 128, NK*384) in [p, (k, d)] order; the (k,p)->row unscramble rides
    the same gather."""
    out = np.empty((B, CL, 4 * D), dtype=np.float32)
    out[:, :, 0:D] = c
    bidx = np.arange(B)[:, None]
    if FATOUT:
        dev_out = np.ascontiguousarray(
            dev_out.reshape(B, 128, NK, 3 * D).transpose(0, 2, 1, 3)
        ).reshape(B, CL, 3 * D)
    out[bidx, PERM, D:] = dev_out.astype(np.float32)
    return out


def kernel(c, q, c_mask, q_mask, c_weight, q_weight, cq_weight, bias):
    global LAST_RESULTS
    c = np.ascontiguousarray(np.asarray(c, dtype=np.float32))
    in_maps = _prep(c, q, c_mask, q_mask, c_weight, q_weight, cq_weight, bias)
    os.environ["BASS_NEVER_TRACE"] = "1"  # no NTFF hook in this container
    nc = _build_graph()
    nc.finalize()
    res = run_bass_kernel_spmd(nc, in_maps, core_ids=list(range(N_CORES)))
    LAST_RESULTS = (nc, in_maps)
    dev = np.concatenate([res.results[i]["out"] for i in range(N_CORES)], axis=0)
    return _assemble(c, dev)



# revision 60
# speedup vs baseline: 1.2031x; 1.2031x over previous
"""C2Q (BiDAF-style) attention kernel for 8 TRN2 NeuronCores (v5, HW-tuned).

Pure data parallel: 64 batches sharded 8-per-core. Per batch b (reference):
    S = c @ c_w + (q @ q_w)^T + (c * cq_w) @ q^T + bias      (1024, 128)
    S1 = masked_softmax(S, q_mask, axis=j)
    S2 = masked_softmax(S1, c_mask, axis=i)
    A = S1 @ q ; Bm = S1 @ (S2^T @ c)
    out = [c | A | c*A | c*Bm]                                (1024, 512)

Key algebra: softmax over j is invariant to per-i constants, so c @ c_w
and the bias cancel in S1. Only R[j] = q @ q_w + log-mask(q_mask)
survives as a per-partition bias in the transposed domain.

v2/v3 structure (aimed at the DMA, HWDGE-per-DMA and ACT/DVE rooflines):
  * Device writes ONLY [A | c*A | c*Bm] in fp16 (6MB/core instead of
    16MB f32); the host pastes the exact f32 `c` block and upcasts.
  * c_mask folded multiplicatively: rcprowm = rcprow * m zeroes masked
    rows of S1 BEFORE the second exp, making them exp(0)=1; an exact
    rank-1 correction (ones ⊗ K, K = host-computed masked-row sums of
    [c|1]) is subtracted inside the Traw PSUM accumulation. This keeps
    ONE unmasked cN slab for both the Traw matmul and the elementwise
    c*A / c*Bm products, and drops the per-chunk bias from the G exp.
  * rowsum via 8 one-column PE matmuls (E0T_k^T @ ones).
  * ONE input DMA per batch ([qmod|ones|q|Ts|cT|cN] slab) and ONE
    output DMA per batch (rearranged AP) — HWDGE costs ~625ns per DMA
    instruction, so 19 DMAs/core instead of 91.
  * Stage-pipelined emission: block b emits stA(b+1) | abmm(b-1) woven
    with stB(b+1) | recip_g(b+1) | stC(b) | load(b+2), so the in-order
    PE queue fills ab-rotation waits with next-batch transposes/minis.
  * Output staging (Pool cannot touch PSUM on real HW): per chunk ONE
    psum evacuation with the rcprow scale (DVE, 2 chunks on ACT) writes
    [Bs | As] where As doubles as the A output block; then SBUF-only
    c*A on Pool and c*Bm on DVE (all-f16 4x mode). The out-DMA's 3D AP
    skips the Bs scratch column.
  * Timing loop uses For_i(staggered_reset=True): the inter-iteration
    semaphore reset is staggered instead of a full all-engine barrier,
    letting consecutive iterations overlap (~3.6us/iter on HW).

Device per batch:
    S^T[j,i] = qmodT.T @ cT    (f16, 2 matmuls of N=512)
    E0T      = exp(S^T + R[j])               # ACT bias; bf16 [j, 1024]
    ep_k     = transpose(E0T chunk)          # PE; bf16 psum [i, j]
    rowsum_k = E0T_k^T @ ones                # PE minis -> psum f32
    rcprow = 1/rowsum ; rcprowm = rcprow * cmask
    G_k = exp(ep_k * rcprowm_k)              # ACT scale AP; f16 SBUF
    Traw = -ones^T@K + sum_k G_k^T @ [c_k|1] # psum f32 accum [j, 129]
    Ts = Traw[:,0:128] * (1/Traw[:,128])     # -> f16, into slab
    ab_k = E0T_k^T @ [q | Ts]                # psum f32 [i, 256]
    st = [ab_A*r | c*ab_A*r | c*ab_B*r]      # f16, one DMA per batch
No max-subtraction needed: |S+R| <= ~30 so exp stays in range.
"""

import os
import numpy as np
import ml_dtypes

import concourse.bass as bass
import concourse.tile as tile
from concourse import bacc, mybir
from concourse.bass_utils import run_bass_kernel_spmd

F32 = mybir.dt.float32
F16 = mybir.dt.float16
BF16 = mybir.dt.bfloat16
AF = mybir.ActivationFunctionType
ALU = mybir.AluOpType

N_CORES = 8
B, CL, QL, D = 64, 1024, 128, 128
BPC = B // N_CORES          # batches per core
NK = CL // 128              # 128-row chunks per batch
MASK_NEG = -50.0            # exp(-50+eps) vanishes in f32 sums; in ACT range

# input slab column layout (f16): [qmod | ones | cT | Ts | q | pad | cN].
# One leading DMA covers everything stA needs ([qmod|ones|cT]); Ts is
# device-written scratch (never transferred); [Ts|q] stay adjacent for the
# ab matmul's 256-col moving operand. cN chunks padded to 130 cols so the
# per-chunk c base stays 4B-aligned (keeps DVE/Pool tensor ops in 2x mode
# on HW, whose auto-detect requires packed+aligned reads).
QMOD0, ONES0, CT0, TS0, QROW0, CN0 = 0, 128, 129, 1153, 1281, 1410
CNS = 130                   # padded chunk stride: [c_k | 1 | pad]
SLAB = CN0 + NK * CNS       # 2450

# staging engine assignment knobs (tuned on HW); env-overridable for A/B
def _envset(name, default):
    v = os.environ.get(name)
    return frozenset(int(x) for x in v.split(",") if x != "") if v is not None else default


def _envint(name, default):
    v = os.environ.get(name)
    return int(v) if v is not None else default


EVAC_ACT = _envset("K_EVAC_ACT", frozenset({6, 7}))  # chunks whose evac runs on ACT
CBM_POOL = _envset("K_CBM_POOL", frozenset({0, 1}))  # chunks whose c*Bm runs on Pool
CA_ON_POOL = bool(_envint("K_CA_POOL", 1))         # c*A on Pool (else DVE)
ALT_FROM = _envint("K_ALT_FROM", BPC - 2)          # batches >= this alternate evacs
INTERLEAVE_STC = bool(_envint("K_IL_STC", 0))      # Traw matmuls inside the k-loop
SPLIT_IN_DMA = bool(_envint("K_SPLIT_IN", 1))      # 2 fat input DMAs (else 1)
OUT_PIECES = _envint("K_OUT_PIECES", 2)            # out-DMA pieces (last batch 2x)
FATOUT = bool(_envint("K_FATOUT", 0))              # contiguous [A|cA|cBm] out layout
                                                   # via 2-run evac AP (fat 6KB
                                                   # out-DMA descriptors)
SKIP_IN_DMA = False            # timing probe: skip slab loads
SKIP_OUT_DMA = False           # timing probe: skip out-DMA
NBATCH = BPC                   # timing probe: process only first N batches
STAGGER = True                 # For_i staggered reset (overlap loop iterations)
REPEAT = 1                     # sim probe: straight-line repeats of run_all

LAST_RESULTS = None         # set by kernel() for test.py profiling
M_VEC = [NK] * BPC          # per batch-slot: # of i-chunks with unmasked rows
PERM = None                 # (B, CL) row permutation used by _prep


def _build_graph(loop_n=0):
    """loop_n=0: straight-line graph (production). loop_n=N>0: wrap the whole
    computation in a hardware For_i loop repeating it N times (timing only)."""
    nc = bacc.Bacc()
    IN_LOOP = bool(loop_n)

    in_ext = nc.declare_dram_parameter("inp", [BPC, 128, SLAB], F16, isOutput=False)
    cmR_ext = nc.declare_dram_parameter("cmR", [128, BPC * NK + BPC], F32, isOutput=False)
    nko_ext = nc.declare_dram_parameter("nko", [1, BPC * 129 + 128], F16, isOutput=False)
    id_ext = nc.declare_dram_parameter("ident", [128, 128], BF16, isOutput=False)
    if FATOUT:
        # per-partition-contiguous layout [p, (k, 384)]; host unscrambles
        # (k,p)->row order inside its existing inverse-permutation gather
        out_ext = nc.declare_dram_parameter("out", [BPC, 128, NK * 3 * D], F16, isOutput=True)
    else:
        out_ext = nc.declare_dram_parameter("out", [BPC, CL, 3 * D], F16, isOutput=True)
    HEAD = CN0                  # head tile: [qmod | ones | Ts | q | cT]

    with tile.TileContext(nc) as tc:
        with (
            tc.tile_pool(name="const", bufs=1) as const,
            tc.tile_pool(name="inp", bufs=8) as inp,
            tc.tile_pool(name="e0tp", bufs=5) as e0tp,
            tc.tile_pool(name="gp", bufs=4) as gp,
            tc.tile_pool(name="small", bufs=6) as smallp,
            tc.tile_pool(name="stg", bufs=4) as stg,
            tc.tile_pool(name="stp", bufs=1, space=bass.MemorySpace.PSUM) as stp,
            tc.tile_pool(name="epp", bufs=2, space=bass.MemorySpace.PSUM) as epp,
            tc.tile_pool(name="rsp", bufs=1, space=bass.MemorySpace.PSUM) as rsp,
            tc.tile_pool(name="trawp", bufs=1, space=bass.MemorySpace.PSUM) as trawp,
            tc.tile_pool(name="abp", bufs=2, space=bass.MemorySpace.PSUM) as abp,
        ):
            ident = const.tile([128, 128], BF16, tag="ident")
            cmR = const.tile([128, BPC * NK + BPC], F32, tag="cmR")
            nko = const.tile([1, BPC * 129 + 128], F16, tag="nko")
            warm = const.tile([128, 1], F32, tag="warm")

            def act_warmup():
                # near-dep-free Exp so the ACT table set loads at t=0 instead
                # of stalling the first E0T behind a 1.3us LoadActFuncSet.
                nc.vector.memset(warm[:], 0.0)
                nc.scalar.activation(warm[:], warm[:], AF.Exp)

            def load_consts():
                # emitted after load(0) so batch 0's slab wins the first
                # HWDGE slots; cmR (E0T bias) before ident (transposes).
                nc.sync.dma_start(cmR[:], cmR_ext[:])
                nc.sync.dma_start(ident[:], id_ext[:])
                nc.sync.dma_start(nko[:], nko_ext[:])

            IN = {}
            CN = {}
            E0T = {}
            EP = {}
            RS = {}
            G = {}
            RCP = {}
            ST = {}
            AB2 = {}

            def load(b):
                # ONE slab DMA per batch: the dram block is contiguous, so
                # the DGE emits 128 fat descriptors (~4.9KB each). Splitting
                # this into skinnier pieces measured WORSE on HW (descriptor
                # overhead), even though it simulates faster.
                t = inp.tile([128, SLAB], F16, tag="in")
                if not SKIP_IN_DMA:
                    if SPLIT_IN_DMA:
                        nc.sync.dma_start(t[:, 0:TS0], in_ext[b][:, 0:TS0])
                        nc.sync.dma_start(t[:, QROW0:SLAB], in_ext[b][:, QROW0:SLAB])
                    else:
                        nc.sync.dma_start(t[:], in_ext[b][:])
                IN[b] = t

            def stA(b):
                """S^T + E0T exp."""
                t = IN[b]
                e0t = e0tp.tile([128, CL], BF16, tag="e0t")
                sp = stp.tile([128, CL], F32, tag="sp")
                for h in range(2):
                    nc.tensor.matmul(
                        sp[:, h * 512:(h + 1) * 512], t[:, QMOD0:QMOD0 + 128],
                        t[:, CT0 + h * 512:CT0 + (h + 1) * 512],
                    )
                nc.scalar.activation(
                    e0t[:], sp[:], AF.Exp,
                    bias=cmR[:, BPC * NK + b:BPC * NK + b + 1],
                )
                E0T[b] = e0t

            def transp(b, k):
                if k == 0:
                    EP[b] = epp.tile([128, CL], BF16, tag="ep", name="ep")
                if k >= M_VEC[b]:
                    return
                nc.tensor.transpose(
                    EP[b][:, k * 128:(k + 1) * 128],
                    E0T[b][:, k * 128:(k + 1) * 128], ident[:],
                )

            def mini(b, k):
                if k == 0:
                    RS[b] = rsp.tile([128, NK], F32, tag="rs", name="rs")
                nc.tensor.matmul(
                    RS[b][:, k:k + 1],
                    E0T[b][:, k * 128:(k + 1) * 128], IN[b][:, ONES0:ONES0 + 1],
                )

            def recip_g(b, splits=1):
                """rcprow(+mask) and the M_b G exps (masked chunks dropped).
                splits=2 (ramp only): recip/G in two halves so the first G
                exps start as soon as the early rowsum minis land."""
                rcprow = smallp.tile([128, NK], F32, tag="rcprow")
                rcprowm = smallp.tile([128, NK], F32, tag="rcprowm")
                g = gp.tile([128, CL], F16, tag="g")
                m = M_VEC[b]
                h = (m + 1) // 2 if splits == 2 else NK
                # first piece [0:h], second piece [h:NK]; evac scale needs
                # all NK cols of rcprow, G only the first m
                for lo, hi in ((0, h), (h, NK)):
                    if lo >= hi:
                        continue
                    nc.vector.reciprocal_approx_fast(
                        rcprow[:, lo:hi], RS[b][:, lo:hi]
                    )
                    gm = min(hi, m)
                    if lo < gm:
                        nc.vector.tensor_mul(
                            rcprowm[:, lo:gm], rcprow[:, lo:gm],
                            cmR[:, b * NK + lo:b * NK + gm],
                        )
                    for k in range(lo, gm):
                        nc.scalar.activation(
                            g[:, k * 128:(k + 1) * 128],
                            EP[b][:, k * 128:(k + 1) * 128], AF.Exp,
                            scale=rcprowm[:, k:k + 1],
                        )
                G[b], RCP[b] = g, rcprow

            TRAW = {}

            def stC_init(b):
                """Rank-1 mask-fix matmul opens the Traw accumulation."""
                TRAW[b] = trawp.tile([128, 129], F32, tag="traw", name="traw")[:]
                nc.tensor.matmul(
                    TRAW[b], nko[0:1, BPC * 129:BPC * 129 + 128],
                    nko[0:1, b * 129:(b + 1) * 129],
                    start=True, stop=False,
                )

            def stC_k(b, k):
                if k >= M_VEC[b]:
                    return
                nc.tensor.matmul(
                    TRAW[b], G[b][:, k * 128:(k + 1) * 128],
                    IN[b][:, CN0 + k * CNS:CN0 + k * CNS + 129],
                    start=False, stop=(k == M_VEC[b] - 1),
                )

            def stC_fin(b):
                traw = TRAW[b]
                rcp2 = smallp.tile([128, 1], F32, tag="rcp2")
                nc.vector.reciprocal_approx_fast(rcp2[:], traw[:, 128:129])
                nc.vector.tensor_scalar_mul(
                    IN[b][:, TS0:TS0 + 128], traw[:, 0:128], rcp2[:]
                )

            def stC(b):
                """Traw accumulation (rank-1 mask fix first), then Ts."""
                stC_init(b)
                for k in range(NK):
                    stC_k(b, k)
                stC_fin(b)

            def abmm(b, k):
                """One AB matmul + staging; batched out-DMA on the last.

                ab = [Braw | Araw] (Ts precedes q in the slab). Staging per
                chunk: ONE psum evacuation with the rcprow scale (DVE/ACT;
                Pool cannot touch PSUM) -> st [Bs | As]; As doubles as the
                output A block. Then SBUF-only products c*A (Pool) and
                c*Bm (DVE 4x). Out-DMA skips the Bs scratch column.
                chunk st layout: [Bs | A | c*A | c*Bm] (512 cols)."""
                t, e0t, rcprow = IN[b], E0T[b], RCP[b]
                if k == 0:
                    ST[b] = stg.tile([128, NK * 512], F16, tag="st", name="st")
                st = ST[b]
                ab = abp.tile([128, 2 * QL], F32, tag="ab", name="ab")[:]
                nc.tensor.matmul(
                    ab, e0t[:, k * 128:(k + 1) * 128],
                    t[:, TS0:TS0 + 256],
                )
                cchunk = t[:, CN0 + k * CNS:CN0 + k * CNS + 128]
                evac_act = k in EVAC_ACT or (b >= ALT_FROM and k % 2 == 1)
                if FATOUT:
                    # chunk layout [A|cA|cBm] at k*384 (contiguous for a fat
                    # out-DMA: 128 x 6KB descriptors); Bs scratch banked at
                    # 3072+k*128. The evac dst is a handcrafted 2-run AP:
                    # src ab=[B|A] walks the Bs slot, then jumps (negative
                    # stride) back to the A slot.
                    a0, b0 = k * 384, NK * 384 + k * 128
                    dst = st[:, 2944:2944 + 256].rearrange(
                        "p (a b) -> p a b", b=128
                    ).copy()
                    dst.offset = dst.offset + (b0 - 2944)
                    dap = dst.ap
                    dap[1] = (a0 - b0, 2)
                    dst.ap = dap
                    sB, sA = b0, a0
                    sCA, sCB = a0 + 128, a0 + 256
                    esrc = ab.rearrange("p (a b) -> p a b", b=128)
                else:
                    s0 = k * 512
                    dst = st[:, s0:s0 + 256]
                    sB, sA = s0, s0 + 128
                    sCA, sCB = s0 + 256, s0 + 384
                    esrc = ab
                # evac psum with scale: st[Bs|As] = ab * rcprow_k
                if not evac_act:
                    nc.vector.tensor_scalar_mul(dst, esrc, rcprow[:, k:k + 1])
                else:
                    nc.scalar.activation(
                        dst, esrc, AF.Copy, scale=rcprow[:, k:k + 1],
                    )
                last = b == BPC - 1
                # c*A = As * c  (SBUF-only f16; Pool cannot touch PSUM but can
                # this). Last batch: alternate onto DVE — Pool otherwise
                # serializes the drain while DVE idles.
                ca_pool = CA_ON_POOL and not (last and k % 2 == 1)
                ca_eng = nc.gpsimd if ca_pool else nc.vector
                ca_eng.tensor_mul(
                    st[:, sCA:sCA + 128], st[:, sA:sA + 128], cchunk
                )
                # c*Bm = Bs * c  (SBUF-only f16; split Pool/DVE for balance)
                cbm_eng = nc.gpsimd if (k in CBM_POOL and not last) else nc.vector
                cbm_eng.tensor_mul(
                    st[:, sCB:sCB + 128], st[:, sB:sB + 128], cchunk
                )
                # out-DMA in pieces (2x finer for the last batch) so the
                # drain transfer starts as early as possible
                pieces = min(NK, OUT_PIECES * 2) if last else OUT_PIECES
                per = NK // pieces
                if (k + 1) % per == 0 and not SKIP_OUT_DMA:
                    if FATOUT:
                        c0, c1 = (k + 1 - per) * 384, (k + 1) * 384
                        nc.sync.dma_start(out_ext[b][:, c0:c1], st[:, c0:c1])
                    else:
                        r0, r1 = (k + 1 - per) * 128, (k + 1) * 128
                        nc.sync.dma_start(
                            out_ext[b][r0:r1].rearrange("(k p) d -> p k d", p=128),
                            st[:, (k + 1 - per) * 512:(k + 1) * 512]
                            .rearrange("p (k d) -> p k d", d=512)[:, :, 128:512],
                        )

            def run_all():
                # pipeline: block b emits stA(b+1) | abmm(b-1)⊗stB(b+1) |
                # recip_g(b+1) | stC(b) | load(b+2). In-order PE queue fills
                # ab-rotation and exp(b+1) waits with interleaved work.
                NB = NBATCH
                if not IN_LOOP:
                    act_warmup()
                load(0)
                if not IN_LOOP:
                    load_consts()
                if NB > 1:
                    load(1)
                if NB > 2:
                    load(2)
                stA(0)
                for k in range(NK):
                    transp(0, k)
                    mini(0, k)
                recip_g(0)
                for b in range(NB):
                    interleave = INTERLEAVE_STC or b + 1 >= NB
                    if b + 3 < NB:
                        load(b + 3)
                    if b + 1 < NB:
                        stA(b + 1)
                    for k in range(NK):
                        # Traw(b) interleaved into the k-loop: its PE matmuls
                        # ride along, pulling Ts(b) ~1us earlier so the next
                        # body's ab-chain (and its evacs) never starve.
                        if b >= 1:
                            abmm(b - 1, k)
                        if interleave:
                            if k == 0:
                                stC_init(b)
                            stC_k(b, k)
                        if b + 1 < NB:
                            transp(b + 1, k)
                            mini(b + 1, k)
                    if b + 1 < NB:
                        recip_g(b + 1)
                    if interleave:
                        stC_fin(b)
                    else:
                        stC(b)
                for k in range(NK):
                    abmm(NB - 1, k)

            if loop_n:
                # consts + ACT table warmup hoisted out of the timing loop:
                # they are iteration-invariant (3 DMAs + an exp per pass).
                act_warmup()
                load_consts()
                with tc.For_i(0, loop_n, 1, staggered_reset=STAGGER):
                    run_all()
            else:
                for _ in range(REPEAT):
                    run_all()
    return nc


def _prep(c, q, c_mask, q_mask, c_weight, q_weight, cq_weight, bias):
    global M_VEC, PERM
    c = np.ascontiguousarray(np.asarray(c, dtype=np.float32))
    q = np.ascontiguousarray(np.asarray(q, dtype=np.float32))
    c_mask = np.asarray(c_mask)
    q_mask = np.asarray(q_mask)
    q_weight = np.asarray(q_weight, dtype=np.float32)
    cq_weight = np.asarray(cq_weight, dtype=np.float32)
    f16 = np.float16

    # host-side prep (tiny). NOTE: c@c_weight and bias cancel in softmax_j.
    s1 = (q.reshape(-1, D) @ q_weight).reshape(B, QL)          # (B, 128)
    R = s1 + np.where(q_mask > 0, 0.0, MASK_NEG).astype(np.float32)
    cm0 = (c_mask > 0)                                         # (B, 1024)

    # Permute rows per batch: unmasked first. The second softmax only
    # involves unmasked rows, so the G/Traw chunk loops shrink to
    # M = ceil(max_unmasked/128) chunks (max across the 8 cores sharing
    # the graph); fully-masked chunks drop out exactly (their would-be
    # G=1 contribution and its rank-1 correction cancel). Exact for ANY
    # input since the graph is built after seeing the masks.
    PERM = np.argsort(~cm0, axis=1, kind="stable")             # (B, CL)
    bidx = np.arange(B)[:, None]
    c = np.ascontiguousarray(c[bidx, PERM])
    cm = cm0[bidx, PERM].astype(np.float32)
    n_unm = cm0.sum(axis=1).astype(np.int64)                   # (B,)
    per_slot = n_unm.reshape(N_CORES, BPC).max(axis=0)         # (BPC,)
    M_VEC = [max(1, min(NK, -(-int(n) // 128))) for n in per_slot]

    cT = c.transpose(0, 2, 1).astype(f16)                      # (B, 128, 1024)
    qmodT = (q * cq_weight.reshape(1, 1, D)).transpose(0, 2, 1).astype(f16)
    # cN: natural chunks [c_k | 1 | pad] -> (B, 128, NK*CNS)
    cNc = c.reshape(B, NK, 128, D).transpose(0, 2, 1, 3)       # (B, p, k, d)
    cN = np.concatenate(
        [cNc, np.ones((B, 128, NK, 1), np.float32),
         np.zeros((B, 128, NK, CNS - 129), np.float32)], axis=3
    ).astype(f16).reshape(B, 128, NK * CNS)
    slab = np.concatenate(
        [qmodT, np.ones((B, 128, 1), f16), cT, np.zeros((B, 128, 128), f16),
         q.astype(f16), np.zeros((B, 128, CN0 - QROW0 - 128), f16), cN],
        axis=2,
    )                                                          # (B, 128, SLAB)
    # rank-1 mask correction: K = [sum c | count] over the masked rows that
    # remain inside the first M chunks (rows beyond M*128 never enter Traw)
    w = (1.0 - cm)                                             # masked rows
    for b in range(B):
        mb = M_VEC[b % BPC]
        w[b, mb * 128:] = 0.0
    Kc = np.einsum('bi,bid->bd', w, c)                         # (B, 128)
    Kn = w.sum(axis=1)                                         # (B,)
    negK = -np.concatenate([Kc, Kn[:, None]], axis=1)          # (B, 129)

    in_maps = []
    for core in range(N_CORES):
        sl = slice(core * BPC, (core + 1) * BPC)
        cmN = cm[sl].reshape(BPC, NK, 128).transpose(2, 0, 1).reshape(128, BPC * NK)
        cmR = np.ascontiguousarray(
            np.concatenate([cmN, R[sl].T], axis=1)             # (128, 64+8)
        )
        nko = np.concatenate(
            [negK[sl].reshape(1, BPC * 129), np.ones((1, 128), np.float32)],
            axis=1,
        ).astype(f16)
        in_maps.append({
            "inp": np.ascontiguousarray(slab[sl]),
            "cmR": cmR,
            "nko": np.ascontiguousarray(nko),
            "ident": np.eye(128, dtype=ml_dtypes.bfloat16),
        })
    return in_maps


def make_in_maps():
    """For the local test/compare harness only (imports reference)."""
    import reference
    inputs = {k: np.asarray(v) for k, v in reference.setup_inputs().items()}
    return _prep(**inputs)


def _assemble(c, dev_out):
    """dev_out f16, rows in PERM order -> full (B, CL, 512) f32 with rows
    unpermuted and the exact c block pasted. FATOUT: dev_out is
    (B, 128, NK*384) in [p, (k, d)] order; the (k,p)->row unscramble rides
    the same gather."""
    out = np.empty((B, CL, 4 * D), dtype=np.float32)
    out[:, :, 0:D] = c
    bidx = np.arange(B)[:, None]
    if FATOUT:
        dev_out = np.ascontiguousarray(
            dev_out.reshape(B, 128, NK, 3 * D).transpose(0, 2, 1, 3)
        ).reshape(B, CL, 3 * D)
    out[bidx, PERM, D:] = dev_out.astype(np.float32)
    return out


def kernel(c, q, c_mask, q_mask, c_weight, q_weight, cq_weight, bias):
    global LAST_RESULTS
    c = np.ascontiguousarray(np.asarray(c, dtype=np.float32))
    in_maps = _prep(c, q, c_mask, q_mask, c_weight, q_weight, cq_weight, bias)
    os.environ["BASS_NEVER_TRACE"] = "1"  # no NTFF hook in this container
    nc = _build_graph()
    nc.finalize()
    res = run_bass_kernel_spmd(nc, in_maps, core_ids=list(range(N_CORES)))
    LAST_RESULTS = (nc, in_maps)
    dev = np.concatenate([res.results[i]["out"] for i in range(N_CORES)], axis=0)
    return _assemble(c, dev)

